# revision 11
# baseline (speedup 1.0000x reference)
"""Multi-head attention (B=2, S=2048, D=1024, H=16) on 8 Trainium2 cores.

Sharding: core c handles batch c//4 and head-group c%4 (4 heads x dk 64).
Q/K/V/O projection weights are column-split by head group on the host.
Attention is computed head-locally with a transposed-scores layout; the
softmax denominator comes from a ones-column folded into the V operand.

Optimizations over the first version:
- Score matmuls for the two heads of a pair (SBUF partitions 0-63 /
  64-127) are interleaved so the PE row-tiles (0,0)/(64,0) run
  concurrently (K=64 each, full-array utilization).
- One [128, 1024] PSUM score tile per (k-tile, q-half) holds both heads
  side by side -> one FD-1024 exp per tile (double-buffered, bufs=2).
- PV matmuls for k-tile kt-1 are emitted between the score matmuls of
  kt, keeping PE busy while ACT runs exp.
- PSUM uses all 8 banks for attention (scores 2x2 + pv 2x2); the
  projection / transpose / output-projection PSUM needs are served from
  the same two pools (phase-disjoint), no separate mm pool.
- All projection biases are folded into the matmul accumulation groups
  via ones-row rank-1 matmuls; PSUM evacuation moved to the vector
  engine, freeing the scalar engine for exp.
- Input-tile DMA double buffered (xpool bufs=48).

Per-head outputs are exchanged with one AllGather per 512-token chunk
inside each 4-core batch group, and each core runs the output projection
for its own token chunk (selected with a partition-id dynamic slice).
"""

import numpy as np
import ml_dtypes

import concourse.bass as bass
import concourse.tile as tile
from concourse import bacc, mybir
from concourse.bass_utils import run_bass_kernel_spmd

BF16 = mybir.dt.bfloat16
F32 = mybir.dt.float32
NPBF16 = ml_dtypes.bfloat16

B, S, D, H = 2, 2048, 1024, 16
DK = 64
N_CORES = 8
HPC = 4               # heads per core
FEAT = HPC * DK       # 256 projected features per core
VW = HPC * (DK + 1)   # 260: v with a ones column per head
TOKC = 512            # token chunk for projections
QCH = 1024            # q chunk for attention
NKC = D // 128        # 8 contraction chunks
CHUNK = S // 4        # 512-token output chunk per core
NKT = S // 128        # 16 k tiles

_CACHE = {}


def _build_program():
    if "nc" in _CACHE:
        return _CACHE["nc"]

    from concourse.masks import make_identity

    nc = bacc.Bacc("TRN2", target_bir_lowering=False, debug=False,
                   num_devices=N_CORES)

    xq = nc.declare_dram_parameter("xq", [D, S], BF16, isOutput=False)
    xk = nc.declare_dram_parameter("xk", [D, S], BF16, isOutput=False)
    xv = nc.declare_dram_parameter("xv", [D, S], BF16, isOutput=False)
    wq = nc.declare_dram_parameter("wq", [D, FEAT], BF16, isOutput=False)
    wk = nc.declare_dram_parameter("wk", [D, FEAT], BF16, isOutput=False)
    wv = nc.declare_dram_parameter("wv", [D, VW], BF16, isOutput=False)
    wo = nc.declare_dram_parameter("wo", [D, D], BF16, isOutput=False)
    bq = nc.declare_dram_parameter("bq", [1, FEAT], BF16, isOutput=False)
    bk = nc.declare_dram_parameter("bk", [1, FEAT], BF16, isOutput=False)
    bv = nc.declare_dram_parameter("bv", [1, VW], BF16, isOutput=False)
    bo = nc.declare_dram_parameter("bo", [1, D], BF16, isOutput=False)
    out = nc.declare_dram_parameter("out", [D, CHUNK], F32, isOutput=True)

    with tile.TileContext(nc) as tc:
        with (
            tc.tile_pool(name="w", bufs=1) as wpool,
            tc.tile_pool(name="x", bufs=48) as xpool,
            tc.tile_pool(name="qk", bufs=1) as qkpool,
            tc.tile_pool(name="vp", bufs=1) as vpool,
            tc.tile_pool(name="sc", bufs=8) as scpool,
            tc.tile_pool(name="sm", bufs=8) as smpool,
            tc.tile_pool(name="cat", bufs=1) as catpool,
            tc.tile_pool(name="fo", bufs=3) as fopool,
            tc.tile_pool(name="ps_sc", bufs=2, space="PSUM") as ps_sc,
            tc.tile_pool(name="ps_pv", bufs=1, space="PSUM") as ps_pv,
            tc.tile_pool(name="dram", bufs=1, space="DRAM") as dram,
        ):
            # ---- constants / weights -------------------------------------
            ident = wpool.tile([128, 128], BF16, tag="ident")
            make_identity(nc, ident[:])
            ones1 = wpool.tile([1, 128], BF16, tag="ones")
            nc.vector.memset(ones1[:], 1.0)
            ones_tok = wpool.tile([1, TOKC], BF16, tag="onest")
            nc.vector.memset(ones_tok[:], 1.0)

            wq_sb = []
            wk_sb = []
            wv_sb = []
            wo_sb = []
            for kc in range(NKC):
                t = wpool.tile([128, FEAT], BF16, tag=f"wq{kc}")
                nc.sync.dma_start(t[:], wq[bass.ts(kc, 128), :])
                wq_sb.append(t)
                t = wpool.tile([128, FEAT], BF16, tag=f"wk{kc}")
                nc.sync.dma_start(t[:], wk[bass.ts(kc, 128), :])
                wk_sb.append(t)
                t = wpool.tile([128, VW], BF16, tag=f"wv{kc}")
                nc.sync.dma_start(t[:], wv[bass.ts(kc, 128), :])
                wv_sb.append(t)
                t = wpool.tile([128, D], BF16, tag=f"wo{kc}")
                nc.sync.dma_start(t[:], wo[bass.ts(kc, 128), :])
                wo_sb.append(t)
            bq_sb = wpool.tile([1, FEAT], BF16, tag="bq")
            nc.sync.dma_start(bq_sb[:], bq[:])
            bk_sb = wpool.tile([1, FEAT], BF16, tag="bk")
            nc.sync.dma_start(bk_sb[:], bk[:])
            bv_sb = wpool.tile([1, VW], BF16, tag="bv")
            nc.sync.dma_start(bv_sb[:], bv[:])
            bo_sb = wpool.tile([1, D], BF16, tag="bo")
            nc.sync.dma_start(bo_sb[:], bo[:])

            # ---- phase 1: projections ------------------------------------
            # qh/kh transposed: [feat, tok]; v natural: [tok, dk+1 per head]
            qh_sb = [qkpool.tile([128, S], BF16, tag=f"qh{m}", name=f"qh{m}")
                     for m in range(2)]
            kh_sb = [qkpool.tile([128, S], BF16, tag=f"kh{m}", name=f"kh{m}")
                     for m in range(2)]
            v_sb = [vpool.tile([128, VW], BF16, tag=f"v{j}", name=f"v{j}")
                    for j in range(S // 128)]

            for t0 in range(S // TOKC):
                tok = bass.ts(t0, TOKC)
                xq_t, xk_t, xv_t = [], [], []
                for kc in range(NKC):
                    t = xpool.tile([128, TOKC], BF16, tag="xt")
                    nc.sync.dma_start(t[:], xq[bass.ts(kc, 128), tok])
                    xq_t.append(t)
                for kc in range(NKC):
                    t = xpool.tile([128, TOKC], BF16, tag="xt")
                    nc.sync.dma_start(t[:], xk[bass.ts(kc, 128), tok])
                    xk_t.append(t)
                for kc in range(NKC):
                    t = xpool.tile([128, TOKC], BF16, tag="xt")
                    nc.sync.dma_start(t[:], xv[bass.ts(kc, 128), tok])
                    xv_t.append(t)

                for (w_sb, x_t, b_sb, dst) in (
                    (wq_sb, xq_t, bq_sb, qh_sb),
                    (wk_sb, xk_t, bk_sb, kh_sb),
                ):
                    for m in range(2):
                        ps = ps_sc.tile([128, TOKC], F32, tag="sc")
                        for kc in range(NKC):
                            nc.tensor.matmul(
                                ps[:], w_sb[kc][:, bass.ts(m, 128)], x_t[kc][:],
                                start=(kc == 0), stop=False,
                            )
                        nc.tensor.matmul(
                            ps[:], b_sb[:, bass.ts(m, 128)], ones_tok[:],
                            start=False, stop=True,
                        )
                        nc.vector.tensor_copy(dst[m][:, tok], ps[:])

                for j in range(TOKC // 128):
                    ps = ps_pv.tile([128, VW], F32, tag=f"pv{j % 2}",
                                    name="psv")
                    for kc in range(NKC):
                        nc.tensor.matmul(
                            ps[:], xv_t[kc][:, bass.ts(j, 128)], wv_sb[kc][:],
                            start=(kc == 0), stop=False,
                        )
                    nc.tensor.matmul(ps[:], ones1[:], bv_sb[:],
                                     start=False, stop=True)
                    nc.vector.tensor_copy(v_sb[t0 * 4 + j][:], ps[:])

            # ---- phase 2: attention --------------------------------------
            ag_in = [dram.tile([FEAT, CHUNK], BF16, tag=f"agi{c}", name=f"agi{c}")
                     for c in range(4)]
            ag_out = dram.tile([4 * D, CHUNK], BF16, tag="ago")

            for qch in range(S // QCH):
                for pr in range(2):
                    # pv accumulators for the two heads of this pair
                    pv = [ps_pv.tile([128, 1024], F32, tag=f"pv{a}",
                                     name=f"pv{a}")
                          for a in range(2)]
                    sc_tiles = [[None, None] for _ in range(NKT)]

                    def emit_pv(k):
                        for a in range(2):
                            h = 2 * pr + a
                            for j in range(QCH // 128):
                                u, jj = j // 4, j % 4
                                nc.tensor.matmul(
                                    pv[a][:, j * 128:j * 128 + DK + 1],
                                    sc_tiles[k][u][:,
                                                   a * 512 + jj * 128:
                                                   a * 512 + (jj + 1) * 128],
                                    v_sb[k][:, h * (DK + 1):(h + 1) * (DK + 1)],
                                    # one bank-clear per PSUM bank, first
                                    # k-tile only; later j-groups overwrite
                                    # into cleared territory, later k-tiles
                                    # accumulate
                                    start=(k == 0 and j % 4 == 0),
                                    stop=(k == NKT - 1),
                                )

                    for kt in range(NKT):
                        for u in range(QCH // 512):
                            q0 = qch * QCH + u * 512
                            ps = ps_sc.tile([128, 1024], F32, tag="sc")
                            # interleave the two heads' score matmuls: row
                            # tiles (0,0) / (64,0) run concurrently on PE
                            for a, base in ((0, 0), (1, 64)):
                                nc.tensor.matmul(
                                    ps[:, a * 512:(a + 1) * 512],
                                    kh_sb[pr][base:base + 64, bass.ts(kt, 128)],
                                    qh_sb[pr][base:base + 64, q0:q0 + 512],
                                    start=True, stop=True,
                                )
                            sc = scpool.tile([128, 1024], BF16, tag="sc",
                                             name="sc")
                            nc.scalar.activation(
                                sc[:], ps[:],
                                mybir.ActivationFunctionType.Exp,
                                scale=0.125,
                            )
                            sc_tiles[kt][u] = sc
                        if kt > 0:
                            emit_pv(kt - 1)
                    emit_pv(NKT - 1)

                    # ---- normalize + transpose + stage for AllGather -----
                    for a in range(2):
                        h = 2 * pr + a
                        pvs = smpool.tile([128, 1024], F32, tag="pvs",
                                          name="pvs")
                        nc.vector.tensor_copy(pvs[:], pv[a][:])
                        for j in range(QCH // 128):
                            r = smpool.tile([128, 1], F32, tag="r")
                            nc.vector.reciprocal(
                                r[:], pvs[:, j * 128 + DK:j * 128 + DK + 1])
                            onrm = smpool.tile([128, DK], BF16, tag="onrm")
                            nc.vector.tensor_scalar_mul(
                                onrm[:], pvs[:, j * 128:j * 128 + DK], r[:])
                            tp = ps_pv.tile([DK, 128], BF16, tag=f"pv{a}",
                                            name="tp")
                            nc.tensor.transpose(tp[:], onrm[:], ident[:])
                            tpsb = smpool.tile([DK, 128], BF16, tag="tp")
                            nc.vector.tensor_copy(tpsb[:], tp[:])
                            q0 = qch * QCH + j * 128
                            nc.sync.dma_start(
                                ag_in[q0 // CHUNK][h * DK:(h + 1) * DK,
                                                   q0 % CHUNK:q0 % CHUNK + 128],
                                tpsb[:],
                            )
                # fire the AllGathers for the two finished 512-chunks
                for c in (2 * qch, 2 * qch + 1):
                    nc.gpsimd.collective_compute(
                        "AllGather", mybir.AluOpType.bypass,
                        replica_groups=[[0, 1, 2, 3], [4, 5, 6, 7]],
                        ins=[ag_in[c].opt()],
                        outs=[ag_out[c * D:(c + 1) * D, :].opt()],
                    )

            # ---- phase 3: output projection for my token chunk -----------
            pid = nc.sync.partition_id()
            base = nc.sync.snap((pid % 4) * D, donate=False,
                                min_val=0, max_val=3 * D)
            cat_sb = []
            for kc in range(NKC):
                t = catpool.tile([128, CHUNK], BF16, tag=f"cat{kc}")
                nc.sync.dma_start(
                    t[:], ag_out[bass.ds(base + kc * 128, 128), :])
                cat_sb.append(t)
            for m in range(NKC):
                ps = ps_sc.tile([128, CHUNK], F32, tag="sc")
                for kc in range(NKC):
                    nc.tensor.matmul(
                        ps[:], wo_sb[kc][:, bass.ts(m, 128)], cat_sb[kc][:],
                        start=(kc == 0), stop=False,
                    )
                nc.tensor.matmul(
                    ps[:], bo_sb[:, bass.ts(m, 128)], ones_tok[:],
                    start=False, stop=True,
                )
                fo = fopool.tile([128, CHUNK], F32, tag="fo")
                nc.vector.tensor_copy(fo[:], ps[:])
                nc.sync.dma_start(out[bass.ts(m, 128), :], fo[:])

    nc.compile()
    _CACHE["nc"] = nc
    return nc


def _prep_inputs(q, k, v, Wq, bq, Wk, bk, Wv, bv, Wo, bo):
    """Build the per-core input maps (host-side sharding)."""
    woT = np.ascontiguousarray(Wo.T).astype(NPBF16)
    bo_r = bo.reshape(1, D).astype(NPBF16)
    in_maps = []
    for c in range(N_CORES):
        b, hg = c // 4, c % 4
        fsl = slice(FEAT * hg, FEAT * (hg + 1))
        wv_aug = np.zeros((D, VW), np.float32)
        bv_aug = np.zeros((VW,), np.float32)
        for h in range(HPC):
            rows = slice(FEAT * hg + DK * h, FEAT * hg + DK * (h + 1))
            wv_aug[:, h * (DK + 1):h * (DK + 1) + DK] = Wv[rows, :].T
            bv_aug[h * (DK + 1):h * (DK + 1) + DK] = bv[rows]
            bv_aug[h * (DK + 1) + DK] = 1.0
        in_maps.append({
            "xq": np.ascontiguousarray(q[b].T).astype(NPBF16),
            "xk": np.ascontiguousarray(k[b].T).astype(NPBF16),
            "xv": np.ascontiguousarray(v[b].T).astype(NPBF16),
            "wq": np.ascontiguousarray(Wq[fsl].T).astype(NPBF16),
            "wk": np.ascontiguousarray(Wk[fsl].T).astype(NPBF16),
            "wv": wv_aug.astype(NPBF16),
            "wo": woT,
            "bq": bq[fsl].reshape(1, FEAT).astype(NPBF16),
            "bk": bk[fsl].reshape(1, FEAT).astype(NPBF16),
            "bv": bv_aug.reshape(1, VW).astype(NPBF16),
            "bo": bo_r,
        })
    return in_maps


def run_sharded(in_maps, trace=False):
    nc = _build_program()
    res = run_bass_kernel_spmd(nc, in_maps, list(range(N_CORES)), trace=trace)
    full = np.empty((B, S, D), np.float32)
    for c in range(N_CORES):
        b, blk = c // 4, c % 4
        full[b, CHUNK * blk:CHUNK * (blk + 1), :] = res.results[c]["out"].T
    return full, res


def kernel(q, k, v, Wq, bq, Wk, bk, Wv, bv, Wo, bo):
    args = [np.asarray(x, np.float32) for x in
            (q, k, v, Wq, bq, Wk, bk, Wv, bv, Wo, bo)]
    in_maps = _prep_inputs(*args)
    full, _ = run_sharded(in_maps)
    return full


# revision 16
# speedup vs baseline: 32.8482x; 32.8482x over previous
"""Multi-head attention (B=2, S=2048, D=1024, H=16) on 8 Trainium2 cores.

Sharding: core c handles batch c//4 and head-group c%4 (4 heads x dk 64).
Q/K/V/O projection weights are column-split by head group on the host.
Attention is computed head-locally with a transposed-scores layout; the
softmax denominator comes from a ones-column folded into the V operand.

Optimizations over the first version:
- Score matmuls for the two heads of a pair (SBUF partitions 0-63 /
  64-127) are interleaved so the PE row-tiles (0,0)/(64,0) run
  concurrently (K=64 each, full-array utilization).
- One [128, 1024] PSUM score tile per (k-tile, q-half) holds both heads
  side by side -> one FD-1024 exp per tile (double-buffered, bufs=2).
- PV matmuls for k-tile kt-1 are emitted between the score matmuls of
  kt, keeping PE busy while ACT runs exp.
- PSUM uses all 8 banks for attention (scores 2x2 + pv 2x2); the
  projection / transpose / output-projection PSUM needs are served from
  the same two pools (phase-disjoint), no separate mm pool.
- All projection biases are folded into the matmul accumulation groups
  via ones-row rank-1 matmuls; PSUM evacuation moved to the vector
  engine, freeing the scalar engine for exp.
- Input-tile DMA double buffered (xpool bufs=48).

Per-head outputs are exchanged with one AllGather per 512-token chunk
inside each 4-core batch group, and each core runs the output projection
for its own token chunk (selected with a partition-id dynamic slice).
"""

import numpy as np
import ml_dtypes

import concourse.bass as bass
import concourse.tile as tile
from concourse import bacc, mybir
from concourse.bass_utils import run_bass_kernel_spmd

BF16 = mybir.dt.bfloat16
F32 = mybir.dt.float32
NPBF16 = ml_dtypes.bfloat16

B, S, D, H = 2, 2048, 1024, 16
DK = 64
N_CORES = 8
HPC = 4               # heads per core
FEAT = HPC * DK       # 256 projected features per core
VW = HPC * (DK + 1)   # 260: v with a ones column per head
TOKC = 512            # token chunk for projections
QCH = 1024            # q chunk for attention
NKC = D // 128        # 8 contraction chunks
CHUNK = S // 4        # 512-token output chunk per core
NKT = S // 128        # 16 k tiles

_CACHE = {}


def _build_program():
    if "nc" in _CACHE:
        return _CACHE["nc"]

    from concourse.masks import make_identity

    nc = bacc.Bacc("TRN2", target_bir_lowering=False, debug=False,
                   num_devices=N_CORES)

    xq = nc.declare_dram_parameter("xq", [D, S], BF16, isOutput=False)
    xk = nc.declare_dram_parameter("xk", [D, S], BF16, isOutput=False)
    xv = nc.declare_dram_parameter("xv", [D, S], BF16, isOutput=False)
    wq = nc.declare_dram_parameter("wq", [D, FEAT], BF16, isOutput=False)
    wk = nc.declare_dram_parameter("wk", [D, FEAT], BF16, isOutput=False)
    wv = nc.declare_dram_parameter("wv", [D, VW], BF16, isOutput=False)
    wo = nc.declare_dram_parameter("wo", [D, D], BF16, isOutput=False)
    bq = nc.declare_dram_parameter("bq", [1, FEAT], BF16, isOutput=False)
    bk = nc.declare_dram_parameter("bk", [1, FEAT], BF16, isOutput=False)
    bv = nc.declare_dram_parameter("bv", [1, VW], BF16, isOutput=False)
    bo = nc.declare_dram_parameter("bo", [1, D], BF16, isOutput=False)
    out = nc.declare_dram_parameter("out", [D, CHUNK], F32, isOutput=True)

    with tile.TileContext(nc) as tc:
        with (
            tc.tile_pool(name="w", bufs=1) as wpool,
            tc.tile_pool(name="x", bufs=48) as xpool,
            tc.tile_pool(name="qk", bufs=1) as qkpool,
            tc.tile_pool(name="vp", bufs=1) as vpool,
            tc.tile_pool(name="sc", bufs=8) as scpool,
            tc.tile_pool(name="sm", bufs=8) as smpool,
            tc.tile_pool(name="cat", bufs=1) as catpool,
            tc.tile_pool(name="fo", bufs=3) as fopool,
            tc.tile_pool(name="ps_sc", bufs=2, space="PSUM") as ps_sc,
            tc.tile_pool(name="ps_pv", bufs=1, space="PSUM") as ps_pv,
            tc.tile_pool(name="dram", bufs=1, space="DRAM") as dram,
        ):
            # ---- constants / weights -------------------------------------
            ident = wpool.tile([128, 128], BF16, tag="ident")
            make_identity(nc, ident[:])
            ones1 = wpool.tile([1, 128], BF16, tag="ones")
            nc.vector.memset(ones1[:], 1.0)
            ones_tok = wpool.tile([1, TOKC], BF16, tag="onest")
            nc.vector.memset(ones_tok[:], 1.0)

            wq_sb = []
            wk_sb = []
            wv_sb = []
            wo_sb = []
            for kc in range(NKC):
                t = wpool.tile([128, FEAT], BF16, tag=f"wq{kc}")
                nc.sync.dma_start(t[:], wq[bass.ts(kc, 128), :])
                wq_sb.append(t)
                t = wpool.tile([128, FEAT], BF16, tag=f"wk{kc}")
                nc.sync.dma_start(t[:], wk[bass.ts(kc, 128), :])
                wk_sb.append(t)
                t = wpool.tile([128, VW], BF16, tag=f"wv{kc}")
                nc.sync.dma_start(t[:], wv[bass.ts(kc, 128), :])
                wv_sb.append(t)
                t = wpool.tile([128, D], BF16, tag=f"wo{kc}")
                nc.sync.dma_start(t[:], wo[bass.ts(kc, 128), :])
                wo_sb.append(t)
            bq_sb = wpool.tile([1, FEAT], BF16, tag="bq")
            nc.sync.dma_start(bq_sb[:], bq[:])
            bk_sb = wpool.tile([1, FEAT], BF16, tag="bk")
            nc.sync.dma_start(bk_sb[:], bk[:])
            bv_sb = wpool.tile([1, VW], BF16, tag="bv")
            nc.sync.dma_start(bv_sb[:], bv[:])
            bo_sb = wpool.tile([1, D], BF16, tag="bo")
            nc.sync.dma_start(bo_sb[:], bo[:])

            # ---- phase 1: projections ------------------------------------
            # qh/kh transposed: [feat, tok]; v natural: [tok, dk+1 per head]
            qh_sb = [qkpool.tile([128, S], BF16, tag=f"qh{m}", name=f"qh{m}")
                     for m in range(2)]
            kh_sb = [qkpool.tile([128, S], BF16, tag=f"kh{m}", name=f"kh{m}")
                     for m in range(2)]
            v_sb = [vpool.tile([128, VW], BF16, tag=f"v{j}", name=f"v{j}")
                    for j in range(S // 128)]

            def load_x(t0):
                tok = bass.ts(t0, TOKC)
                lists = []
                for src in (xq, xk, xv):
                    lst = []
                    for kc in range(NKC):
                        t = xpool.tile([128, TOKC], BF16, tag="xt",
                                       name="xt")
                        nc.sync.dma_start(t[:], src[bass.ts(kc, 128), tok])
                        lst.append(t)
                    lists.append(lst)
                return lists

            def qk_group(w_sb, x_t, b_sb, dst, m, t0):
                ps = ps_sc.tile([128, TOKC], F32, tag="sc", name="psqk")
                for kc in range(NKC):
                    nc.tensor.matmul(
                        ps[:], w_sb[kc][:, bass.ts(m, 128)], x_t[kc][:],
                        start=(kc == 0), stop=False,
                    )
                nc.tensor.matmul(
                    ps[:], b_sb[:, bass.ts(m, 128)], ones_tok[:],
                    start=False, stop=True,
                )
                nc.vector.tensor_copy(dst[m][:, bass.ts(t0, TOKC)], ps[:])

            def v_group(x_t, t0, j, in_attention):
                # during the attention interleave the pv slots hold live
                # accumulators; borrow the sc ring instead (no deadlock:
                # its ring only waits on exp reads, which keep flowing)
                if in_attention:
                    ps = ps_sc.tile([128, VW], F32, tag="sc", name="psv")
                else:
                    ps = ps_pv.tile([128, VW], F32, tag=f"pv{j % 2}",
                                    name="psv")
                for kc in range(NKC):
                    nc.tensor.matmul(
                        ps[:], x_t[kc][:, bass.ts(j, 128)], wv_sb[kc][:],
                        start=(kc == 0), stop=False,
                    )
                nc.tensor.matmul(ps[:], ones1[:], bv_sb[:],
                                 start=False, stop=True)
                nc.vector.tensor_copy(v_sb[t0 * 4 + j][:], ps[:])

            # chunks 0,1 projected up front; chunks 2,3 are interleaved
            # into the first attention pair's k-tile windows below
            xts = {}
            for t0 in (0, 1):
                xts[t0] = load_x(t0)
                for m in range(2):
                    qk_group(wq_sb, xts[t0][0], bq_sb, qh_sb, m, t0)
                for m in range(2):
                    qk_group(wk_sb, xts[t0][1], bk_sb, kh_sb, m, t0)
                for j in range(TOKC // 128):
                    v_group(xts[t0][2], t0, j, False)
            xts[2] = load_x(2)
            xts[3] = load_x(3)
            pending = []
            for t0 in (2, 3):
                for m in range(2):
                    pending.append(
                        (qk_group, (wk_sb, xts[t0][1], bk_sb, kh_sb, m, t0)))
                for j in range(TOKC // 128):
                    pending.append((v_group, (xts[t0][2], t0, j, True)))
            for t0 in (2, 3):
                for m in range(2):
                    pending.append(
                        (qk_group, (wq_sb, xts[t0][0], bq_sb, qh_sb, m, t0)))
            pending.reverse()  # pop() from the front

            # ---- phase 2: attention --------------------------------------
            ag_in = [dram.tile([FEAT, CHUNK], BF16, tag=f"agi{c}", name=f"agi{c}")
                     for c in range(4)]
            ag_out = dram.tile([4 * D, CHUNK], BF16, tag="ago")

            for qch in range(S // QCH):
                for pr in range(2):
                    # pv accumulators for the two heads of this pair
                    pv = [ps_pv.tile([128, 1024], F32, tag=f"pv{a}",
                                     name=f"pv{a}")
                          for a in range(2)]
                    sc_tiles = [[None, None] for _ in range(NKT)]

                    def emit_pv(k):
                        for a in range(2):
                            h = 2 * pr + a
                            for j in range(QCH // 128):
                                u, jj = j // 4, j % 4
                                nc.tensor.matmul(
                                    pv[a][:, j * 128:j * 128 + DK + 1],
                                    sc_tiles[k][u][:,
                                                   a * 512 + jj * 128:
                                                   a * 512 + (jj + 1) * 128],
                                    v_sb[k][:, h * (DK + 1):(h + 1) * (DK + 1)],
                                    # one bank-clear per PSUM bank, first
                                    # k-tile only; later j-groups overwrite
                                    # into cleared territory, later k-tiles
                                    # accumulate
                                    start=(k == 0 and j % 4 == 0),
                                    stop=(k == NKT - 1),
                                )

                    for kt in range(NKT):
                        for u in range(QCH // 512):
                            q0 = qch * QCH + u * 512
                            ps = ps_sc.tile([128, 1024], F32, tag="sc")
                            # interleave the two heads' score matmuls: row
                            # tiles (0,0) / (64,0) run concurrently on PE
                            for a, base in ((0, 0), (1, 64)):
                                nc.tensor.matmul(
                                    ps[:, a * 512:(a + 1) * 512],
                                    kh_sb[pr][base:base + 64, bass.ts(kt, 128)],
                                    qh_sb[pr][base:base + 64, q0:q0 + 512],
                                    start=True, stop=True,
                                )
                            sc = scpool.tile([128, 1024], BF16, tag="sc",
                                             name="sc")
                            nc.scalar.activation(
                                sc[:], ps[:],
                                mybir.ActivationFunctionType.Exp,
                                scale=0.125,
                            )
                            sc_tiles[kt][u] = sc
                        if kt > 0:
                            emit_pv(kt - 1)
                        if pending:
                            fn, fargs = pending.pop()
                            fn(*fargs)
                    emit_pv(NKT - 1)

                    # ---- normalize + transpose + stage for AllGather -----
                    # evacuate both heads' accumulators first, then
                    # alternate transposes across the two pv PSUM tags so
                    # the PE-transpose / DVE-copy chain is 2-deep instead
                    # of serial (copying first also avoids a PE<->DVE
                    # ordering cycle on the pv1 ring)
                    pvs_t = []
                    for a in range(2):
                        pvs = smpool.tile([128, 1024], F32, tag=f"pvs{a}",
                                          name="pvs")
                        nc.vector.tensor_copy(pvs[:], pv[a][:])
                        pvs_t.append(pvs)
                    tcount = 0
                    for a in range(2):
                        h = 2 * pr + a
                        pvs = pvs_t[a]
                        for j in range(QCH // 128):
                            r = smpool.tile([128, 1], F32, tag="r")
                            nc.vector.reciprocal(
                                r[:], pvs[:, j * 128 + DK:j * 128 + DK + 1])
                            onrm = smpool.tile([128, DK], BF16, tag="onrm")
                            nc.vector.tensor_scalar_mul(
                                onrm[:], pvs[:, j * 128:j * 128 + DK], r[:])
                            tp = ps_pv.tile([DK, 128], BF16,
                                            tag=f"pv{tcount % 2}", name="tp")
                            tcount += 1
                            nc.tensor.transpose(tp[:], onrm[:], ident[:])
                            tpsb = smpool.tile([DK, 128], BF16, tag="tp")
                            nc.vector.tensor_copy(tpsb[:], tp[:])
                            q0 = qch * QCH + j * 128
                            nc.sync.dma_start(
                                ag_in[q0 // CHUNK][h * DK:(h + 1) * DK,
                                                   q0 % CHUNK:q0 % CHUNK + 128],
                                tpsb[:],
                            )
                # fire the AllGathers for the two finished 512-chunks
                for c in (2 * qch, 2 * qch + 1):
                    nc.gpsimd.collective_compute(
                        "AllGather", mybir.AluOpType.bypass,
                        replica_groups=[[0, 1, 2, 3], [4, 5, 6, 7]],
                        ins=[ag_in[c].opt()],
                        outs=[ag_out[c * D:(c + 1) * D, :].opt()],
                    )

            # ---- phase 3: output projection for my token chunk -----------
            pid = nc.sync.partition_id()
            base = nc.sync.snap((pid % 4) * D, donate=False,
                                min_val=0, max_val=3 * D)
            cat_sb = []
            for kc in range(NKC):
                t = catpool.tile([128, CHUNK], BF16, tag=f"cat{kc}")
                nc.sync.dma_start(
                    t[:], ag_out[bass.ds(base + kc * 128, 128), :])
                cat_sb.append(t)
            for m in range(NKC):
                ps = ps_sc.tile([128, CHUNK], F32, tag="sc")
                for kc in range(NKC):
                    nc.tensor.matmul(
                        ps[:], wo_sb[kc][:, bass.ts(m, 128)], cat_sb[kc][:],
                        start=(kc == 0), stop=False,
                    )
                nc.tensor.matmul(
                    ps[:], bo_sb[:, bass.ts(m, 128)], ones_tok[:],
                    start=False, stop=True,
                )
                fo = fopool.tile([128, CHUNK], F32, tag="fo")
                nc.vector.tensor_copy(fo[:], ps[:])
                nc.sync.dma_start(out[bass.ts(m, 128), :], fo[:])

    nc.compile()
    _CACHE["nc"] = nc
    return nc


def _prep_inputs(q, k, v, Wq, bq, Wk, bk, Wv, bv, Wo, bo):
    """Build the per-core input maps (host-side sharding)."""
    woT = np.ascontiguousarray(Wo.T).astype(NPBF16)
    bo_r = bo.reshape(1, D).astype(NPBF16)
    in_maps = []
    for c in range(N_CORES):
        b, hg = c // 4, c % 4
        fsl = slice(FEAT * hg, FEAT * (hg + 1))
        wv_aug = np.zeros((D, VW), np.float32)
        bv_aug = np.zeros((VW,), np.float32)
        for h in range(HPC):
            rows = slice(FEAT * hg + DK * h, FEAT * hg + DK * (h + 1))
            wv_aug[:, h * (DK + 1):h * (DK + 1) + DK] = Wv[rows, :].T
            bv_aug[h * (DK + 1):h * (DK + 1) + DK] = bv[rows]
            bv_aug[h * (DK + 1) + DK] = 1.0
        in_maps.append({
            "xq": np.ascontiguousarray(q[b].T).astype(NPBF16),
            "xk": np.ascontiguousarray(k[b].T).astype(NPBF16),
            "xv": np.ascontiguousarray(v[b].T).astype(NPBF16),
            "wq": np.ascontiguousarray(Wq[fsl].T).astype(NPBF16),
            "wk": np.ascontiguousarray(Wk[fsl].T).astype(NPBF16),
            "wv": wv_aug.astype(NPBF16),
            "wo": woT,
            "bq": bq[fsl].reshape(1, FEAT).astype(NPBF16),
            "bk": bk[fsl].reshape(1, FEAT).astype(NPBF16),
            "bv": bv_aug.reshape(1, VW).astype(NPBF16),
            "bo": bo_r,
        })
    return in_maps


def run_sharded(in_maps, trace=False):
    nc = _build_program()
    res = run_bass_kernel_spmd(nc, in_maps, list(range(N_CORES)), trace=trace)
    full = np.empty((B, S, D), np.float32)
    for c in range(N_CORES):
        b, blk = c // 4, c % 4
        full[b, CHUNK * blk:CHUNK * (blk + 1), :] = res.results[c]["out"].T
    return full, res


def kernel(q, k, v, Wq, bq, Wk, bk, Wv, bv, Wo, bo):
    args = [np.asarray(x, np.float32) for x in
            (q, k, v, Wq, bq, Wk, bk, Wv, bv, Wo, bo)]
    in_maps = _prep_inputs(*args)
    full, _ = run_sharded(in_maps)
    return full
